# revision 1
# baseline (speedup 1.0000x reference)
"""Trainium2 kernel for nn_DP_53755810677168 (DeePMD-style GNN message passing).

Sharding strategy (per sharding_hint): data-parallel over the natoms axis.
The 2048-atom axis is split into 8 contiguous blocks of 256 atoms; each
NeuronCore owns one block together with its full neighbor rows (ImageDR
slice) and a replicated copy of the small per-type-pair embedding/fitting
weights.  Per-atom energies are computed fully on-device; the host only
scatters the 8 shards back together and reduces Etot = sum_n Ei.
"""

import numpy as np
import jax
import jax.numpy as jnp

NTYPES = 2
RMIN = 0.5
RMAX = 6.0
M2 = 16
EMB_LAST = 100
N_CORES = 8
N_ATOMS = 2048
B = 4


def _emb_forward(x, p):
    for W, b in zip(p['W'], p['b']):
        h = jnp.tanh(x @ W + b)
        din, dout = W.shape
        if dout == din:
            x = h + x
        elif dout == 2 * din:
            x = h + jnp.concatenate([x, x], axis=-1)
        else:
            x = h
    return x


def _fit_forward(x, p):
    x = jnp.tanh(x @ p['W0'] + p['b0'])
    x = x + jnp.tanh(x @ p['W1'] + p['b1'])
    return x @ p['W2'] + p['b2']


def _shard_fn(ImageDR, tmap, davg, dstd, params):
    # ImageDR: [B, N/8, 200, 4]; tmap: [N/8]
    Bs, N, M, _ = ImageDR.shape
    m = M // NTYPES
    R = ImageDR[..., 0]
    safeR = jnp.where(jnp.abs(R) > 1e-5, R, 1.0)
    u = (R - RMIN) / (RMAX - RMIN)
    sw = (u ** 3) * (-6.0 * u * u + 15.0 * u - 10.0) + 1.0
    Srij = jnp.where((R > 0) & (R < RMIN), 1.0 / safeR,
                     jnp.where((R > RMIN) & (R < RMAX), sw / safeR, 0.0))
    mask = (jnp.abs(R) > 1e-5)[..., None]
    Rxyz = jnp.where(mask, Srij[..., None] * ImageDR[..., 1:4] / safeR[..., None], 0.0)
    Ri = jnp.concatenate([Srij[..., None], Rxyz], axis=-1)
    davg_t = davg[tmap].reshape(1, N, M, 4)
    dstd_t = dstd[tmap].reshape(1, N, M, 4)
    Ri = (Ri - davg_t) / dstd_t
    S = Ri[..., 0:1]
    Ei = jnp.zeros((Bs, N, 1), ImageDR.dtype)
    for i in range(NTYPES):
        xyz = jnp.zeros((Bs, N, 4, EMB_LAST), ImageDR.dtype)
        for j in range(NTYPES):
            sl = slice(j * m, (j + 1) * m)
            G = _emb_forward(S[:, :, sl, :], params['emb'][i][j])
            xyz = xyz + jnp.einsum('bnmf,bnme->bnfe', Ri[:, :, sl, :], G)
        xyz = xyz / M
        DR = jnp.einsum('bnfe,bnfk->bnek', xyz, xyz[..., :M2]).reshape(Bs, N, -1)
        Ei_i = _fit_forward(DR, params['fit'][i])
        Ei = Ei + jnp.where((tmap == i)[None, :, None], Ei_i, 0.0)
    return Ei[..., 0]


_PMAPPED = None


def _get_pmapped():
    global _PMAPPED
    if _PMAPPED is None:
        devs = jax.devices()[:N_CORES]
        _PMAPPED = jax.pmap(_shard_fn, in_axes=(0, 0, None, None, None),
                            devices=devs)
    return _PMAPPED


def kernel(list_neigh, Imagetype_map, ImageDR, davg, dstd, params):
    del list_neigh  # only needed for the force-scatter path, not energies
    f = _get_pmapped()
    ns = N_ATOMS // N_CORES
    dr = np.asarray(ImageDR)
    tm = np.asarray(Imagetype_map)
    dr_s = np.stack(np.split(dr, N_CORES, axis=1))          # [8, B, 256, 200, 4]
    tm_s = np.stack(np.split(tm, N_CORES))                  # [8, 256]
    Ei_s = f(dr_s, tm_s, jnp.asarray(davg), jnp.asarray(dstd), params)
    Ei = np.transpose(np.asarray(Ei_s), (1, 0, 2)).reshape(B, N_ATOMS)
    Etot = Ei.sum(axis=1, keepdims=True).astype(np.float32)
    return jnp.asarray(Etot), jnp.asarray(Ei)


# revision 2
# speedup vs baseline: 1.0542x; 1.0542x over previous
"""Trainium2 kernel for nn_DP_53755810677168 (DeePMD-style GNN message passing).

Sharding strategy (per sharding_hint): data-parallel over the natoms axis,
refined by atom type.  Atoms are permuted on the host so that cores 0-3 own
only type-0 atoms and cores 4-7 own only type-1 atoms (each core a
contiguous padded block with its full neighbor rows).  Because the
per-type-pair embedding / fitting weights are passed per-shard, every core
runs the SAME program but evaluates only the ONE center-type branch its
atoms need — halving the dominant embedding-MLP work vs. the reference's
compute-both-and-mask formulation.  The small weights are replicated
(sharded copies), per-atom energies are computed fully on-device, and the
host only scatters the 8 shards back and reduces Etot = sum_n Ei.
"""

import numpy as np
import jax
import jax.numpy as jnp

NTYPES = 2
RMIN = 0.5
RMAX = 6.0
M2 = 16
EMB_LAST = 100
N_CORES = 8
N_ATOMS = 2048
B = 4


def _emb_forward(x, p):
    for W, b in zip(p['W'], p['b']):
        h = jnp.tanh(x @ W + b)
        din, dout = W.shape
        if dout == din:
            x = h + x
        elif dout == 2 * din:
            x = h + jnp.concatenate([x, x], axis=-1)
        else:
            x = h
    return x


def _fit_forward(x, p):
    x = jnp.tanh(x @ p['W0'] + p['b0'])
    x = x + jnp.tanh(x @ p['W1'] + p['b1'])
    return x @ p['W2'] + p['b2']


def _shard_fn(ImageDR, davg_row, dstd_row, emb_p, fit_p):
    # ImageDR: [B, ns, 200, 4] — all atoms of ONE type on this core.
    # davg_row/dstd_row: [800] for that type; emb_p: [j=0,1] params; fit_p: params.
    Bs, N, M, _ = ImageDR.shape
    m = M // NTYPES
    R = ImageDR[..., 0]
    safeR = jnp.where(jnp.abs(R) > 1e-5, R, 1.0)
    u = (R - RMIN) / (RMAX - RMIN)
    sw = (u ** 3) * (-6.0 * u * u + 15.0 * u - 10.0) + 1.0
    Srij = jnp.where((R > 0) & (R < RMIN), 1.0 / safeR,
                     jnp.where((R > RMIN) & (R < RMAX), sw / safeR, 0.0))
    mask = (jnp.abs(R) > 1e-5)[..., None]
    Rxyz = jnp.where(mask, Srij[..., None] * ImageDR[..., 1:4] / safeR[..., None], 0.0)
    Ri = jnp.concatenate([Srij[..., None], Rxyz], axis=-1)          # [B,ns,M,4]
    Ri = (Ri - davg_row.reshape(1, 1, M, 4)) / dstd_row.reshape(1, 1, M, 4)
    S = Ri[..., 0:1]
    xyz = jnp.zeros((Bs, N, 4, EMB_LAST), ImageDR.dtype)
    for j in range(NTYPES):
        sl = slice(j * m, (j + 1) * m)
        G = _emb_forward(S[:, :, sl, :], emb_p[j])                  # [B,ns,m,100]
        xyz = xyz + jnp.einsum('bnmf,bnme->bnfe', Ri[:, :, sl, :], G)
    xyz = xyz / M
    DR = jnp.einsum('bnfe,bnfk->bnek', xyz, xyz[..., :M2]).reshape(Bs, N, -1)
    Ei = _fit_forward(DR, fit_p)                                    # [B,ns,1]
    return Ei[..., 0]


_PMAPPED = None


def _get_pmapped():
    global _PMAPPED
    if _PMAPPED is None:
        devs = jax.devices()[:N_CORES]
        _PMAPPED = jax.pmap(_shard_fn, in_axes=(0, 0, 0, 0, 0), devices=devs)
    return _PMAPPED


def kernel(list_neigh, Imagetype_map, ImageDR, davg, dstd, params):
    del list_neigh  # only needed for the force-scatter path, not energies
    f = _get_pmapped()
    dr = np.asarray(ImageDR)
    tm = np.asarray(Imagetype_map)
    davg = np.asarray(davg)
    dstd = np.asarray(dstd)

    # Host-side sharding: 4 cores per atom type, contiguous index chunks.
    chunks, lens = [], []
    for t in range(NTYPES):
        idx_t = np.where(tm == t)[0]
        for part in np.array_split(idx_t, N_CORES // NTYPES):
            chunks.append(part)
            lens.append(len(part))
    ns_pad = max(lens)
    chunks_p = [np.concatenate([c, np.full(ns_pad - len(c), c[0], c.dtype)])
                for c in chunks]

    dr_s = np.stack([dr[:, c] for c in chunks_p])                   # [8,B,ns,200,4]
    davg_s = np.stack([davg[0 if i < 4 else 1] for i in range(N_CORES)])
    dstd_s = np.stack([dstd[0 if i < 4 else 1] for i in range(N_CORES)])
    emb_s = jax.tree.map(
        lambda *xs: np.stack([np.asarray(x) for x in xs]),
        *[params['emb'][0 if i < 4 else 1] for i in range(N_CORES)])
    fit_s = jax.tree.map(
        lambda *xs: np.stack([np.asarray(x) for x in xs]),
        *[params['fit'][0 if i < 4 else 1] for i in range(N_CORES)])

    Ei_s = np.asarray(f(dr_s, davg_s, dstd_s, emb_s, fit_s))        # [8,B,ns]
    Ei = np.empty((B, N_ATOMS), np.float32)
    for c, (chunk, ln) in enumerate(zip(chunks, lens)):
        Ei[:, chunk] = Ei_s[c][:, :ln]
    Etot = Ei.sum(axis=1, keepdims=True).astype(np.float32)
    return jnp.asarray(Etot), jnp.asarray(Ei)


# revision 3
# speedup vs baseline: 60.5085x; 57.3963x over previous
"""Trainium2 kernel for nn_DP_53755810677168 (DeePMD-style GNN message passing).

Sharding strategy (per sharding_hint): data-parallel over the natoms axis,
refined by atom type.  Atoms are permuted on the host so that cores 0-3 own
only type-0 atoms and cores 4-7 own only type-1 atoms (each core a
contiguous padded block with its full neighbor rows).  Because the
per-type-pair embedding / fitting weights are passed per-shard, every core
runs the SAME program but evaluates only the ONE center-type branch its
atoms need — halving the dominant embedding-MLP work vs. the reference's
compute-both-and-mask formulation.  The small weights are replicated
(sharded copies), per-atom energies are computed fully on-device, and the
host only scatters the 8 shards back and reduces Etot = sum_n Ei.
"""

import numpy as np
import jax
import jax.numpy as jnp

NTYPES = 2
RMIN = 0.5
RMAX = 6.0
M2 = 16
EMB_LAST = 100
N_CORES = 8
N_ATOMS = 2048
B = 4


def _emb_forward(x, p):
    for W, b in zip(p['W'], p['b']):
        h = jnp.tanh(x @ W + b)
        din, dout = W.shape
        if dout == din:
            x = h + x
        elif dout == 2 * din:
            x = h + jnp.concatenate([x, x], axis=-1)
        else:
            x = h
    return x


def _fit_forward(x, p):
    x = jnp.tanh(x @ p['W0'] + p['b0'])
    x = x + jnp.tanh(x @ p['W1'] + p['b1'])
    return x @ p['W2'] + p['b2']


def _shard_fn(ImageDR, davg_row, dstd_row, emb_p, fit_p):
    # ImageDR: [B, ns, 200, 4] — all atoms of ONE type on this core.
    # davg_row/dstd_row: [800] for that type; emb_p: [j=0,1] params; fit_p: params.
    Bs, N, M, _ = ImageDR.shape
    m = M // NTYPES
    R = ImageDR[..., 0]
    safeR = jnp.where(jnp.abs(R) > 1e-5, R, 1.0)
    u = (R - RMIN) / (RMAX - RMIN)
    sw = (u ** 3) * (-6.0 * u * u + 15.0 * u - 10.0) + 1.0
    Srij = jnp.where((R > 0) & (R < RMIN), 1.0 / safeR,
                     jnp.where((R > RMIN) & (R < RMAX), sw / safeR, 0.0))
    mask = (jnp.abs(R) > 1e-5)[..., None]
    Rxyz = jnp.where(mask, Srij[..., None] * ImageDR[..., 1:4] / safeR[..., None], 0.0)
    Ri = jnp.concatenate([Srij[..., None], Rxyz], axis=-1)          # [B,ns,M,4]
    Ri = (Ri - davg_row.reshape(1, 1, M, 4)) / dstd_row.reshape(1, 1, M, 4)
    S = Ri[..., 0:1]
    xyz = jnp.zeros((Bs, N, 4, EMB_LAST), ImageDR.dtype)
    for j in range(NTYPES):
        sl = slice(j * m, (j + 1) * m)
        G = _emb_forward(S[:, :, sl, :], emb_p[j])                  # [B,ns,m,100]
        xyz = xyz + jnp.einsum('bnmf,bnme->bnfe', Ri[:, :, sl, :], G)
    xyz = xyz / M
    DR = jnp.einsum('bnfe,bnfk->bnek', xyz, xyz[..., :M2]).reshape(Bs, N, -1)
    Ei = _fit_forward(DR, fit_p)                                    # [B,ns,1]
    return Ei[..., 0]


_PMAPPED = None


def _get_pmapped():
    global _PMAPPED
    if _PMAPPED is None:
        devs = jax.devices()[:N_CORES]
        _PMAPPED = jax.pmap(_shard_fn, in_axes=(0, 0, 0, 0, 0), devices=devs)
    return _PMAPPED


def _shard_host(Imagetype_map, ImageDR, davg, dstd, params):
    """Host-side sharding: 4 cores per atom type, contiguous index chunks."""
    dr = np.asarray(ImageDR)
    tm = np.asarray(Imagetype_map)
    davg = np.asarray(davg)
    dstd = np.asarray(dstd)

    chunks, lens = [], []
    for t in range(NTYPES):
        idx_t = np.where(tm == t)[0]
        for part in np.array_split(idx_t, N_CORES // NTYPES):
            chunks.append(part)
            lens.append(len(part))
    ns_pad = max(lens)
    chunks_p = [np.concatenate([c, np.full(ns_pad - len(c), c[0], c.dtype)])
                for c in chunks]

    dr_s = np.stack([dr[:, c] for c in chunks_p])                   # [8,B,ns,200,4]
    davg_s = np.stack([davg[0 if i < 4 else 1] for i in range(N_CORES)])
    dstd_s = np.stack([dstd[0 if i < 4 else 1] for i in range(N_CORES)])
    emb_s = jax.tree.map(
        lambda *xs: np.stack([np.asarray(x) for x in xs]),
        *[params['emb'][0 if i < 4 else 1] for i in range(N_CORES)])
    fit_s = jax.tree.map(
        lambda *xs: np.stack([np.asarray(x) for x in xs]),
        *[params['fit'][0 if i < 4 else 1] for i in range(N_CORES)])
    return (dr_s, davg_s, dstd_s, emb_s, fit_s), chunks, lens


def shard_inputs(list_neigh, Imagetype_map, ImageDR, davg, dstd, params):
    """Shard + place on the 8 cores (for device-resident benchmarking)."""
    del list_neigh
    args, _, _ = _shard_host(Imagetype_map, ImageDR, davg, dstd, params)
    devs = jax.devices()[:N_CORES]
    put = lambda x: jax.device_put_sharded([x[i] for i in range(N_CORES)], devs)
    return jax.tree.map(put, args)


def kernel(list_neigh, Imagetype_map, ImageDR, davg, dstd, params):
    del list_neigh  # only needed for the force-scatter path, not energies
    f = _get_pmapped()
    args, chunks, lens = _shard_host(Imagetype_map, ImageDR, davg, dstd, params)
    Ei_s = np.asarray(f(*args))                                     # [8,B,ns]
    Ei = np.empty((B, N_ATOMS), np.float32)
    for c, (chunk, ln) in enumerate(zip(chunks, lens)):
        Ei[:, chunk] = Ei_s[c][:, :ln]
    Etot = Ei.sum(axis=1, keepdims=True).astype(np.float32)
    return jnp.asarray(Etot), jnp.asarray(Ei)


# revision 5
# speedup vs baseline: 67.9475x; 1.1229x over previous
"""Trainium2 kernel for nn_DP_53755810677168 (DeePMD-style GNN message passing).

Sharding strategy (per sharding_hint): data-parallel over the natoms axis,
refined by atom type.  Atoms are permuted on the host so that cores 0-3 own
only type-0 atoms and cores 4-7 own only type-1 atoms (each core a
contiguous padded block with its full neighbor rows).  Because the
per-type-pair embedding / fitting weights are passed per-shard, every core
runs the SAME program but evaluates only the ONE center-type branch its
atoms need — halving the dominant embedding-MLP work vs. the reference's
compute-both-and-mask formulation.  The small weights are replicated
(sharded copies), per-atom energies are computed fully on-device, and the
host only scatters the 8 shards back and reduces Etot = sum_n Ei.
"""

import numpy as np
import jax
import jax.numpy as jnp

NTYPES = 2
RMIN = 0.5
RMAX = 6.0
M2 = 16
EMB_LAST = 100
N_CORES = 8
N_ATOMS = 2048
B = 4


def _emb_forward(x, p):
    for W, b in zip(p['W'], p['b']):
        h = jnp.tanh(x @ W + b)
        din, dout = W.shape
        if dout == din:
            x = h + x
        elif dout == 2 * din:
            x = h + jnp.concatenate([x, x], axis=-1)
        else:
            x = h
    return x


def _fit_forward(x, p):
    x = jnp.tanh(x @ p['W0'] + p['b0'])
    x = x + jnp.tanh(x @ p['W1'] + p['b1'])
    return x @ p['W2'] + p['b2']


def _shard_fn(ImageDR, davg_row, dstd_row, emb_p, fit_p):
    # ImageDR: [B, ns, 200, 4] — all atoms of ONE type on this core.
    # davg_row/dstd_row: [800] for that type; emb_p: [j=0,1] params; fit_p: params.
    Bs, N, M, _ = ImageDR.shape
    m = M // NTYPES
    R = ImageDR[..., 0]
    safeR = jnp.where(jnp.abs(R) > 1e-5, R, 1.0)
    u = (R - RMIN) / (RMAX - RMIN)
    sw = (u ** 3) * (-6.0 * u * u + 15.0 * u - 10.0) + 1.0
    Srij = jnp.where((R > 0) & (R < RMIN), 1.0 / safeR,
                     jnp.where((R > RMIN) & (R < RMAX), sw / safeR, 0.0))
    mask = (jnp.abs(R) > 1e-5)[..., None]
    Rxyz = jnp.where(mask, Srij[..., None] * ImageDR[..., 1:4] / safeR[..., None], 0.0)
    Ri = jnp.concatenate([Srij[..., None], Rxyz], axis=-1)          # [B,ns,M,4]
    Ri = (Ri - davg_row.reshape(1, 1, M, 4)) / dstd_row.reshape(1, 1, M, 4)
    S = Ri[..., 0:1]
    xyz = jnp.zeros((Bs, N, 4, EMB_LAST), ImageDR.dtype)
    for j in range(NTYPES):
        sl = slice(j * m, (j + 1) * m)
        x2d = S[:, :, sl, :].reshape(-1, 1)                         # [B*ns*m, 1]
        G = _emb_forward(x2d, emb_p[j]).reshape(Bs, N, m, EMB_LAST)
        xyz = xyz + jnp.einsum('bnmf,bnme->bnfe', Ri[:, :, sl, :], G)
    # the two /M factors are folded into fit W0 on the host (W0 / M^2)
    DR = jnp.einsum('bnfe,bnfk->bnek', xyz, xyz[..., :M2]).reshape(Bs, N, -1)
    Ei = _fit_forward(DR, fit_p)                                    # [B,ns,1]
    return Ei[..., 0]


_PMAPPED = None


def _get_pmapped():
    global _PMAPPED
    if _PMAPPED is None:
        devs = jax.devices()[:N_CORES]
        _PMAPPED = jax.pmap(_shard_fn, in_axes=(0, 0, 0, 0, 0), devices=devs)
    return _PMAPPED


def _shard_host(Imagetype_map, ImageDR, davg, dstd, params):
    """Host-side sharding: 4 cores per atom type, contiguous index chunks."""
    dr = np.asarray(ImageDR)
    tm = np.asarray(Imagetype_map)
    davg = np.asarray(davg)
    dstd = np.asarray(dstd)

    chunks, lens = [], []
    for t in range(NTYPES):
        idx_t = np.where(tm == t)[0]
        for part in np.array_split(idx_t, N_CORES // NTYPES):
            chunks.append(part)
            lens.append(len(part))
    ns_pad = max(lens)
    chunks_p = [np.concatenate([c, np.full(ns_pad - len(c), c[0], c.dtype)])
                for c in chunks]

    dr_s = np.stack([dr[:, c] for c in chunks_p])                   # [8,B,ns,200,4]
    davg_s = np.stack([davg[0 if i < 4 else 1] for i in range(N_CORES)])
    dstd_s = np.stack([dstd[0 if i < 4 else 1] for i in range(N_CORES)])
    emb_s = jax.tree.map(
        lambda *xs: np.stack([np.asarray(x) for x in xs]),
        *[params['emb'][0 if i < 4 else 1] for i in range(N_CORES)])
    fit_s = jax.tree.map(
        lambda *xs: np.stack([np.asarray(x) for x in xs]),
        *[params['fit'][0 if i < 4 else 1] for i in range(N_CORES)])
    M = dr.shape[2]
    fit_s = dict(fit_s, W0=fit_s['W0'] / np.float32(M * M))
    return (dr_s, davg_s, dstd_s, emb_s, fit_s), chunks, lens


def shard_inputs(list_neigh, Imagetype_map, ImageDR, davg, dstd, params):
    """Shard + place on the 8 cores (for device-resident benchmarking)."""
    del list_neigh
    args, _, _ = _shard_host(Imagetype_map, ImageDR, davg, dstd, params)
    devs = jax.devices()[:N_CORES]
    put = lambda x: jax.device_put_sharded([x[i] for i in range(N_CORES)], devs)
    return jax.tree.map(put, args)


def kernel(list_neigh, Imagetype_map, ImageDR, davg, dstd, params):
    del list_neigh  # only needed for the force-scatter path, not energies
    f = _get_pmapped()
    args, chunks, lens = _shard_host(Imagetype_map, ImageDR, davg, dstd, params)
    Ei_s = np.asarray(f(*args))                                     # [8,B,ns]
    Ei = np.empty((B, N_ATOMS), np.float32)
    for c, (chunk, ln) in enumerate(zip(chunks, lens)):
        Ei[:, chunk] = Ei_s[c][:, :ln]
    Etot = Ei.sum(axis=1, keepdims=True).astype(np.float32)
    return jnp.asarray(Etot), jnp.asarray(Ei)
